# revision 15
# baseline (speedup 1.0000x reference)
"""GNN edge-softmax attention kernel for Trainium2 (8 NeuronCores).

Strategy (dest-range sharding, two device passes):
  - Core c owns dest-node range [c*6250, (c+1)*6250); edges assigned by dest.
  - Run 1: project q (full) + k (own range) from host-transposed features;
    pass A over edges sorted by src-tile: gather q[src]/k[dest] rows via
    dma_gather, scores -> ex = exp(q.k/4 + mask - 8)  (constant softmax
    shift; exact softmax invariance), segment-sum ex by src via one-hot
    matmul + dynamic-offset accumulate; AllReduce segsum across cores.
  - Host: permute per-edge ex values from pass-A order to pass-B order
    (pure np.take reordering, no arithmetic).
  - Run 2: vn = (v projection) * (1/segsum) per node; pass B over edges
    sorted by dest-tile (src-lo/hi sections for int16 gather indices):
    gather vn[src], weighted = vn * ex, scatter to dest tiles via one-hot
    matmul + dynamic-offset accumulate; final out = acc @ Wo.T + bo.
"""

import os

import numpy as np

import concourse.bacc as bacc
import concourse.bass as bass
import concourse.mybir as mybir
import concourse.tile as tile
from concourse.bass_utils import run_bass_kernel_spmd
from concourse.masks import make_identity

N = 50000
F = 128
H = 8
D = 16
E = 800000
NCORES = 8
RPC = N // NCORES          # 6250 rows per core
KT = (RPC + 127) // 128    # 49 dest tiles per core (last has 106 rows)
NT = (N + 127) // 128      # 391 src tiles (last has 80 rows)
NPAD = NT * 128            # 50048
LO = 32768                 # int16 index boundary
BATCH = 8                  # chunks per dma_gather call (1024 rows)
MASK_PAD = -60.0
SHIFT = 8.0

f32 = mybir.dt.float32
i16 = mybir.dt.int16
i32 = mybir.dt.int32
DVE = mybir.EngineType.DVE
ALU = mybir.AluOpType
ACTF = mybir.ActivationFunctionType


def _wrap_idx(flat):
    """int16 row-index array -> dma_gather wrapped layout [128, len/16]."""
    nb = len(flat) // 1024
    out = np.zeros((128, 64 * nb), np.int16)
    for b in range(nb):
        sl = flat[b * 1024:(b + 1) * 1024].reshape(64, 16).T
        out[:, b * 64:(b + 1) * 64] = np.tile(sl, (8, 1))
    return out


def _bcast16(ap):
    """[128, 8] AP -> [128, 8, 16] with stride-0 inner broadcast."""
    return bass.AP(ap.tensor, ap.offset, list(ap.ap) + [[0, D]])


def _gen_run1(ch_lo, ch_hi):
    CH = ch_lo + ch_hi
    nc = bacc.Bacc(None, target_bir_lowering=False, num_swdge_queues=4)
    nfT = nc.dram_tensor("nfT", [128, NPAD], f32, kind="ExternalInput")
    nfTk = nc.dram_tensor("nfTk", [128, KT * 128], f32, kind="ExternalInput")
    WqT = nc.dram_tensor("WqT", [128, 128], f32, kind="ExternalInput")
    WkT = nc.dram_tensor("WkT", [128, 128], f32, kind="ExternalInput")
    bqk = nc.dram_tensor("bqk", [1, 256], f32, kind="ExternalInput")
    qidx = nc.dram_tensor("qidx", [128, CH * 8], i16, kind="ExternalInput")
    kidx = nc.dram_tensor("kidx", [128, CH * 8], i16, kind="ExternalInput")
    srcloc = nc.dram_tensor("srcloc", [128, CH], f32, kind="ExternalInput")
    segoff = nc.dram_tensor("segoff", [1, CH], i32, kind="ExternalInput")
    maskp = nc.dram_tensor("maskp", [128, CH * H], f32, kind="ExternalInput")
    exout = nc.dram_tensor("exout", [128, CH * H], f32, kind="ExternalOutput")
    segout = nc.dram_tensor("segout", [128, NT * H], f32, kind="ExternalOutput")

    qtab = nc.dram_tensor("qtab", [NPAD, F], f32)
    ktab = nc.dram_tensor("ktab", [KT * 128, F], f32)
    seg_in = nc.dram_tensor("seg_in", [128, NT * H], f32)
    seg_cc = nc.dram_tensor("seg_cc", [128, NT * H], f32)

    with tile.TileContext(nc) as tc:
        with (
            tc.tile_pool(name="meta", bufs=1) as mp,
            tc.tile_pool(name="proj", bufs=3) as pp,
            tc.tile_pool(name="gath", bufs=3) as gp,
            tc.tile_pool(name="work", bufs=4) as wp,
            tc.tile_pool(name="ps", bufs=3, space="PSUM") as ps,
            tc.tile_pool(name="pseg", bufs=4, space="PSUM") as pseg,
        ):
            # ---- constants / metadata loads ----
            wq = mp.tile([128, 128], f32)
            wk = mp.tile([128, 128], f32)
            bias = mp.tile([1, 256], f32)
            ones = mp.tile([1, 128], f32)
            iota = mp.tile([128, 128], f32)
            nc.sync.dma_start(out=wq[:], in_=WqT[:])
            nc.sync.dma_start(out=wk[:], in_=WkT[:])
            nc.sync.dma_start(out=bias[:], in_=bqk[:])
            nc.vector.memset(ones[:], 1.0)
            nc.gpsimd.iota(iota[:], pattern=[[1, 128]], base=0,
                           channel_multiplier=0,
                           allow_small_or_imprecise_dtypes=True)

            qidx_sb = mp.tile([128, CH * 8], i16)
            kidx_sb = mp.tile([128, CH * 8], i16)
            srcloc_sb = mp.tile([128, CH], f32)
            segoff_sb = mp.tile([1, CH], i32)
            mask_sb = mp.tile([128, CH * H], f32)
            nc.sync.dma_start(out=qidx_sb[:], in_=qidx[:])
            nc.sync.dma_start(out=kidx_sb[:], in_=kidx[:])
            nc.sync.dma_start(out=srcloc_sb[:], in_=srcloc[:])
            nc.sync.dma_start(out=segoff_sb[:], in_=segoff[:])
            nc.sync.dma_start(out=mask_sb[:], in_=maskp[:])

            # ---- phase 0: q table (full, scaled by 0.25 via host WqT) ----
            for t in range(NT):
                nft = pp.tile([128, 128], f32, tag="nft")
                nc.sync.dma_start(out=nft[:], in_=nfT[:, t * 128:(t + 1) * 128])
                pq = ps.tile([128, 128], f32, tag="pq")
                nc.tensor.matmul(out=pq[:], lhsT=nft[:], rhs=wq[:],
                                 start=True, stop=False)
                nc.tensor.matmul(out=pq[:], lhsT=ones[:], rhs=bias[:, 0:128],
                                 start=False, stop=True)
                qsb = pp.tile([128, 128], f32, tag="qsb")
                nc.scalar.activation(out=qsb[:], in_=pq[:], func=ACTF.Copy)
                nc.sync.dma_start(out=qtab[t * 128:(t + 1) * 128, :], in_=qsb[:])

            # ---- phase 0b: k table (own dest range, via per-core nfTk) ----
            for t in range(KT):
                nft = pp.tile([128, 128], f32, tag="nft")
                nc.sync.dma_start(out=nft[:], in_=nfTk[:, t * 128:(t + 1) * 128])
                pk = ps.tile([128, 128], f32, tag="pq")
                nc.tensor.matmul(out=pk[:], lhsT=nft[:], rhs=wk[:],
                                 start=True, stop=False)
                nc.tensor.matmul(out=pk[:], lhsT=ones[:], rhs=bias[:, 128:256],
                                 start=False, stop=True)
                ksb = pp.tile([128, 128], f32, tag="qsb")
                nc.scalar.activation(out=ksb[:], in_=pk[:], func=ACTF.Copy)
                nc.sync.dma_start(out=ktab[t * 128:(t + 1) * 128, :], in_=ksb[:])

            # ---- pass A ----
            segacc = mp.tile([128, NT * H], f32)
            nc.vector.memset(segacc[:], 0.0)
            ex_sb = mp.tile([128, CH * H], f32)

            nb_lo = ch_lo // BATCH
            nb = CH // BATCH
            for b in range(nb):
                qg = gp.tile([128, BATCH, F], f32, tag="qg")
                kg = gp.tile([128, BATCH, F], f32, tag="kg")
                if b < nb_lo:
                    qsrc = qtab[0:LO, :]
                else:
                    qsrc = qtab[LO:NPAD, :]
                nc.gpsimd.dma_gather(
                    out_ap=qg[:], in_ap=qsrc,
                    idxs_ap=qidx_sb[:, b * 64:(b + 1) * 64],
                    num_idxs=1024, num_idxs_reg=1024, elem_size=F,
                    queue_num=(2 * b) % 4)
                nc.gpsimd.dma_gather(
                    out_ap=kg[:], in_ap=ktab[:],
                    idxs_ap=kidx_sb[:, b * 64:(b + 1) * 64],
                    num_idxs=1024, num_idxs_reg=1024, elem_size=F,
                    queue_num=(2 * b + 1) % 4)
                for j in range(BATCH):
                    c = b * BATCH + j
                    prod = wp.tile([128, F], f32, tag="prod")
                    nc.vector.tensor_tensor(out=prod[:], in0=qg[:, j, :],
                                            in1=kg[:, j, :], op=ALU.mult)
                    nc.vector.tensor_reduce(
                        out=ex_sb[:, c * H:(c + 1) * H],
                        in_=prod[:].rearrange("p (h d) -> p h d", d=D),
                        axis=mybir.AxisListType.X, op=ALU.add)
                # batch-level: add mask, exp
                sl = slice(b * BATCH * H, (b + 1) * BATCH * H)
                nc.vector.tensor_tensor(out=ex_sb[:, sl], in0=ex_sb[:, sl],
                                        in1=mask_sb[:, sl], op=ALU.add)
                nc.scalar.activation(out=ex_sb[:, sl], in_=ex_sb[:, sl],
                                     func=ACTF.Exp)
                for j in range(BATCH):
                    c = b * BATCH + j
                    oh = wp.tile([128, 128], f32, tag="oh")
                    nc.vector.tensor_scalar(
                        out=oh[:], in0=iota[:], scalar1=srcloc_sb[:, c:c + 1],
                        scalar2=None, op0=ALU.is_equal)
                    psg = pseg.tile([128, H], f32, tag="pseg")
                    nc.tensor.matmul(out=psg[:], lhsT=oh[:],
                                     rhs=ex_sb[:, c * H:(c + 1) * H],
                                     start=True, stop=True)
                    regs = nc.alloc_registers(engines=[DVE])
                    nc.vector.reg_load(regs, segoff_sb[0:1, c:c + 1])
                    off = nc.snap(regs, donate=True, min_val=0,
                                  max_val=(NT - 1) * H)
                    nc.vector.tensor_tensor(
                        out=segacc[:, bass.ds(off, H)],
                        in0=segacc[:, bass.ds(off, H)],
                        in1=psg[:], op=ALU.add)

            nc.sync.dma_start(out=exout[:], in_=ex_sb[:])

            # ---- AllReduce segsum ----
            if os.environ.get("KERNEL_NO_CC"):
                nc.sync.dma_start(out=segout[:], in_=segacc[:])
            else:
                nc.sync.dma_start(out=seg_in[:], in_=segacc[:])
                nc.gpsimd.collective_compute(
                    "AllReduce", ALU.add,
                    replica_groups=[list(range(NCORES))],
                    ins=[seg_in[:]], outs=[seg_cc[:]])
                segf = mp.tile([128, NT * H], f32)
                nc.sync.dma_start(out=segf[:], in_=seg_cc[:])
                nc.sync.dma_start(out=segout[:], in_=segf[:])
    nc.compile()
    return nc


def _gen_run2(ch_lo, ch_hi):
    CH = ch_lo + ch_hi
    nc = bacc.Bacc(None, target_bir_lowering=False, num_swdge_queues=4)
    nfT = nc.dram_tensor("nfT", [128, NPAD], f32, kind="ExternalInput")
    WvT = nc.dram_tensor("WvT", [128, 128], f32, kind="ExternalInput")
    WoT = nc.dram_tensor("WoT", [128, 128], f32, kind="ExternalInput")
    bvo = nc.dram_tensor("bvo", [1, 256], f32, kind="ExternalInput")
    seg = nc.dram_tensor("seg", [128, NT * H], f32, kind="ExternalInput")
    vidx = nc.dram_tensor("vidx", [128, CH * 8], i16, kind="ExternalInput")
    exB = nc.dram_tensor("exB", [128, CH * H], f32, kind="ExternalInput")
    destloc = nc.dram_tensor("destloc", [128, CH], f32, kind="ExternalInput")
    outoff = nc.dram_tensor("outoff", [1, CH], i32, kind="ExternalInput")
    outd = nc.dram_tensor("outd", [RPC, F], f32, kind="ExternalOutput")

    vntab = nc.dram_tensor("vntab", [NPAD, F], f32)

    with tile.TileContext(nc) as tc:
        with (
            tc.tile_pool(name="meta", bufs=1) as mp,
            tc.tile_pool(name="proj", bufs=3) as pp,
            tc.tile_pool(name="gath", bufs=3) as gp,
            tc.tile_pool(name="work", bufs=4) as wp,
            tc.tile_pool(name="ps", bufs=2, space="PSUM") as ps,
            tc.tile_pool(name="pmm", bufs=3, space="PSUM") as pmm,
        ):
            wv = mp.tile([128, 128], f32)
            wo = mp.tile([128, 128], f32)
            bias = mp.tile([1, 256], f32)
            ones = mp.tile([1, 128], f32)
            iota = mp.tile([128, 128], f32)
            ident = mp.tile([128, 128], f32)
            nc.sync.dma_start(out=wv[:], in_=WvT[:])
            nc.sync.dma_start(out=wo[:], in_=WoT[:])
            nc.sync.dma_start(out=bias[:], in_=bvo[:])
            nc.vector.memset(ones[:], 1.0)
            nc.gpsimd.iota(iota[:], pattern=[[1, 128]], base=0,
                           channel_multiplier=0,
                           allow_small_or_imprecise_dtypes=True)
            make_identity(nc, ident[:])

            seg_sb = mp.tile([128, NT * H], f32)
            inv_sb = mp.tile([128, NT * H], f32)
            nc.sync.dma_start(out=seg_sb[:], in_=seg[:])
            nc.vector.tensor_scalar(out=seg_sb[:], in0=seg_sb[:],
                                    scalar1=1e-30, scalar2=None, op0=ALU.max)
            nc.vector.reciprocal(out=inv_sb[:], in_=seg_sb[:])

            vidx_sb = mp.tile([128, CH * 8], i16)
            exB_sb = mp.tile([128, CH * H], f32)
            destloc_sb = mp.tile([128, CH], f32)
            outoff_sb = mp.tile([1, CH], i32)
            nc.sync.dma_start(out=vidx_sb[:], in_=vidx[:])
            nc.sync.dma_start(out=exB_sb[:], in_=exB[:])
            nc.sync.dma_start(out=destloc_sb[:], in_=destloc[:])
            nc.sync.dma_start(out=outoff_sb[:], in_=outoff[:])

            # ---- phase V: vn table = (v projection) * inv_seg ----
            for t in range(NT):
                nft = pp.tile([128, 128], f32, tag="nft")
                nc.sync.dma_start(out=nft[:], in_=nfT[:, t * 128:(t + 1) * 128])
                pv = ps.tile([128, 128], f32, tag="pv")
                nc.tensor.matmul(out=pv[:], lhsT=nft[:], rhs=wv[:],
                                 start=True, stop=False)
                nc.tensor.matmul(out=pv[:], lhsT=ones[:], rhs=bias[:, 0:128],
                                 start=False, stop=True)
                vn = pp.tile([128, 128], f32, tag="vn")
                nc.vector.tensor_tensor(
                    out=vn[:].rearrange("p (h d) -> p h d", d=D),
                    in0=pv[:].rearrange("p (h d) -> p h d", d=D),
                    in1=_bcast16(inv_sb[:, t * H:(t + 1) * H]),
                    op=ALU.mult)
                nc.sync.dma_start(out=vntab[t * 128:(t + 1) * 128, :], in_=vn[:])

            # ---- pass B ----
            acc = mp.tile([128, KT * 128], f32)
            nc.vector.memset(acc[:], 0.0)

            nb_lo = ch_lo // BATCH
            nb = CH // BATCH
            for b in range(nb):
                vg = gp.tile([128, BATCH, F], f32, tag="vg")
                vsrc = vntab[0:LO, :] if b < nb_lo else vntab[LO:NPAD, :]
                nc.gpsimd.dma_gather(
                    out_ap=vg[:], in_ap=vsrc,
                    idxs_ap=vidx_sb[:, b * 64:(b + 1) * 64],
                    num_idxs=1024, num_idxs_reg=1024, elem_size=F,
                    queue_num=b % 4)
                for j in range(BATCH):
                    c = b * BATCH + j
                    wt = wp.tile([128, F], f32, tag="wt")
                    nc.vector.tensor_tensor(
                        out=wt[:].rearrange("p (h d) -> p h d", d=D),
                        in0=vg[:, j, :].rearrange("p (h d) -> p h d", d=D),
                        in1=_bcast16(exB_sb[:, c * H:(c + 1) * H]),
                        op=ALU.mult)
                    oh = wp.tile([128, 128], f32, tag="oh")
                    nc.vector.tensor_scalar(
                        out=oh[:], in0=iota[:],
                        scalar1=destloc_sb[:, c:c + 1],
                        scalar2=None, op0=ALU.is_equal)
                    pm = pmm.tile([128, 128], f32, tag="pm")
                    nc.tensor.matmul(out=pm[:], lhsT=oh[:], rhs=wt[:],
                                     start=True, stop=True)
                    regs = nc.alloc_registers(engines=[DVE])
                    nc.vector.reg_load(regs, outoff_sb[0:1, c:c + 1])
                    off = nc.snap(regs, donate=True, min_val=0,
                                  max_val=(KT - 1) * 128)
                    nc.vector.tensor_tensor(
                        out=acc[:, bass.ds(off, 128)],
                        in0=acc[:, bass.ds(off, 128)],
                        in1=pm[:], op=ALU.add)

            # ---- output projection per dest tile ----
            for t in range(KT):
                rows = min(128, RPC - t * 128)
                pt = ps.tile([128, 128], f32, tag="pv")
                nc.tensor.transpose(out=pt[:], in_=acc[:, t * 128:(t + 1) * 128],
                                    identity=ident[:])
                at = pp.tile([128, 128], f32, tag="at")
                nc.scalar.activation(out=at[:], in_=pt[:], func=ACTF.Copy)
                po = ps.tile([128, 128], f32, tag="po")
                nc.tensor.matmul(out=po[:], lhsT=at[:], rhs=wo[:],
                                 start=True, stop=False)
                nc.tensor.matmul(out=po[:], lhsT=ones[:], rhs=bias[:, 128:256],
                                 start=False, stop=True)
                osb = pp.tile([128, 128], f32, tag="osb")
                nc.scalar.activation(out=osb[:], in_=po[:], func=ACTF.Copy)
                nc.sync.dma_start(out=outd[t * 128:t * 128 + rows, :],
                                  in_=osb[:rows, :])
    nc.compile()
    return nc


def _prep_passA(src, dest, mask, core):
    """Returns per-core pass-A metadata dict + bookkeeping for ex relay."""
    base = core * RPC
    dl = dest - base
    st = src // 128
    order = np.argsort(st, kind="stable")
    src_s, dl_s, mask_s, gid_s = src[order], dl[order], mask[order], order
    lo_chunks = []
    hi_chunks = []
    tile_ids = st[order]
    for t in range(NT):
        lo_i = np.searchsorted(tile_ids, t, side="left")
        hi_i = np.searchsorted(tile_ids, t, side="right")
        cnt = hi_i - lo_i
        if cnt == 0:
            continue
        nch = (cnt + 127) // 128
        for ci in range(nch):
            a = lo_i + ci * 128
            bnd = min(a + 128, hi_i)
            n = bnd - a
            cs = np.zeros(128, np.int64)
            cdl = np.zeros(128, np.int64)
            cm = np.full((128, H), MASK_PAD, np.float32)
            cg = np.full(128, -1, np.int64)
            cs[:n] = src_s[a:bnd]
            cs[n:] = t * 128  # pad src within tile (any row)
            cdl[:n] = dl_s[a:bnd]
            cm[:n] = mask_s[a:bnd] - SHIFT
            cg[:n] = gid_s[a:bnd]
            rec = (t, cs, cdl, cm, cg)
            (lo_chunks if t < LO // 128 else hi_chunks).append(rec)
    return lo_chunks, hi_chunks


def _pad_chunks(chunks, target, tpad):
    while len(chunks) % 1 != 0 or len(chunks) < target:
        cs = np.full(128, tpad * 128, np.int64)
        chunks.append((tpad, cs, np.zeros(128, np.int64),
                       np.full((128, H), MASK_PAD, np.float32),
                       np.full(128, -1, np.int64)))
    return chunks


def _prep_passB(src, dest, core):
    base = core * RPC
    dl = dest - base
    t_of = dl // 128
    is_lo = src < LO
    lo_chunks, hi_chunks = [], []
    for sec, m in ((lo_chunks, is_lo), (hi_chunks, ~is_lo)):
        idx = np.where(m)[0]
        order = idx[np.argsort(t_of[idx], kind="stable")]
        tid = t_of[order]
        for t in range(KT):
            a = np.searchsorted(tid, t, side="left")
            bnd = np.searchsorted(tid, t, side="right")
            cnt = bnd - a
            if cnt == 0:
                continue
            for ci in range((cnt + 127) // 128):
                s = a + ci * 128
                e2 = min(s + 128, bnd)
                n = e2 - s
                csrc = np.zeros(128, np.int64)
                cdl = np.zeros(128, np.int64)
                cg = np.full(128, -1, np.int64)
                eids = order[s:e2]
                csrc[:n] = src[eids]
                csrc[n:] = 0 if sec is lo_chunks else LO
                cdl[:n] = dl[eids] % 128
                cg[:n] = eids
                lo_hi = lo_chunks if sec is lo_chunks else hi_chunks
                lo_hi.append((t, csrc, cdl, cg))
    return lo_chunks, hi_chunks


def _pad_chunksB(chunks, target, lo):
    while len(chunks) < target:
        chunks.append((0, np.full(128, 0 if lo else LO, np.int64),
                       np.zeros(128, np.int64), np.full(128, -1, np.int64)))
    return chunks


def _unwrap_idx(arr):
    """Inverse of _wrap_idx: [128, 64*nb] -> flat row indices."""
    nb = arr.shape[1] // 64
    out = np.zeros(nb * 1024, np.int64)
    for b in range(nb):
        sl = arr[:16, b * 64:(b + 1) * 64]          # [16, 64]
        out[b * 1024:(b + 1) * 1024] = sl.T.reshape(-1)
    return out


def _emulate_run1(in1, ch_lo, ch_hi):
    CH = ch_lo + ch_hi
    results = []
    segsum = np.zeros((128, NT * H), np.float32)
    for m in in1:
        qtab = m["nfT"].T @ m["WqT"] + m["bqk"][0, :128]
        ktab = m["nfTk"].T @ m["WkT"] + m["bqk"][0, 128:]
        qi = _unwrap_idx(m["qidx"])
        ki = _unwrap_idx(m["kidx"])
        ex = np.zeros((128, CH, H), np.float32)
        seg = np.zeros((128, NT * H), np.float32)
        for b in range(CH // BATCH):
            qrows = qi[b * 1024:(b + 1) * 1024]
            if b >= ch_lo // BATCH:
                qrows = qrows + LO
            krows = ki[b * 1024:(b + 1) * 1024]
            qg = qtab[qrows].reshape(BATCH, 128, F).transpose(1, 0, 2)
            kg = ktab[krows].reshape(BATCH, 128, F).transpose(1, 0, 2)
            for j in range(BATCH):
                c = b * BATCH + j
                sc = (qg[:, j, :] * kg[:, j, :]).reshape(128, H, D).sum(-1)
                sc = sc + m["maskp"][:, c * H:(c + 1) * H]
                exc = np.exp(sc)
                ex[:, c] = exc
                srcl = m["srcloc"][:, c].astype(np.int64)
                off = m["segoff"][0, c]
                np.add.at(seg[:, off:off + H].T, (slice(None), srcl), exc.T)
        segsum = segsum + seg
        results.append({"exout": ex.reshape(128, CH * H)})
    for r in results:
        r["segout"] = segsum
    return results


def _emulate_run2(in2, ch_lo, ch_hi):
    CH = ch_lo + ch_hi
    results = []
    for m in in2:
        vtab = m["nfT"].T @ m["WvT"] + m["bvo"][0, :128]
        seg = np.maximum(m["seg"], 1e-30)
        inv = 1.0 / seg
        # vn[node] = v[node] * inv[node%128, (node//128)*H + h]
        invn = inv.reshape(128, NT, H).transpose(1, 0, 2).reshape(NPAD, H)
        vn = vtab.reshape(NPAD, H, D) * invn[:, :, None]
        vn = vn.reshape(NPAD, F)
        vi = _unwrap_idx(m["vidx"])
        exB = m["exB"].reshape(128, CH, H)
        acc = np.zeros((128, KT * 128), np.float32)
        for b in range(CH // BATCH):
            vrows = vi[b * 1024:(b + 1) * 1024]
            if b >= ch_lo // BATCH:
                vrows = vrows + LO
            vg = vn[vrows].reshape(BATCH, 128, F).transpose(1, 0, 2)
            for j in range(BATCH):
                c = b * BATCH + j
                wt = (vg[:, j, :].reshape(128, H, D)
                      * exB[:, c][:, :, None]).reshape(128, F)
                dl = m["destloc"][:, c].astype(np.int64)
                off = m["outoff"][0, c]
                np.add.at(acc[:, off:off + 128].T, (slice(None), dl), wt.T)
        # out projection
        accn = acc.reshape(128, KT, 128).transpose(1, 0, 2).reshape(KT * 128, F)
        out = accn @ m["WoT"] + m["bvo"][0, 128:]
        results.append({"outd": out[:RPC].astype(np.float32)})
    return results


def kernel(node_features, edge_index, attention_mask, Wq, bq, Wk, bk,
           Wv, bv, Wo, bo):
    node_features = np.asarray(node_features, np.float32)
    edge_index = np.asarray(edge_index)
    attention_mask = np.asarray(attention_mask, np.float32)
    Wq, bq = np.asarray(Wq, np.float32), np.asarray(bq, np.float32)
    Wk, bk = np.asarray(Wk, np.float32), np.asarray(bk, np.float32)
    Wv, bv = np.asarray(Wv, np.float32), np.asarray(bv, np.float32)
    Wo, bo = np.asarray(Wo, np.float32), np.asarray(bo, np.float32)

    src_all = edge_index[0].astype(np.int64)
    dest_all = edge_index[1].astype(np.int64)

    nfT = np.zeros((128, NPAD), np.float32)
    nfT[:, :N] = node_features.T
    WqT = np.ascontiguousarray(Wq.T) * 0.25  # fold 1/sqrt(D)
    bq4 = bq * 0.25
    WkT = np.ascontiguousarray(Wk.T)
    WvT = np.ascontiguousarray(Wv.T)
    WoT = np.ascontiguousarray(Wo.T)

    # ---------- per-core edge partition & pass-A prep ----------
    coreA = []
    for c in range(NCORES):
        m = (dest_all // RPC) == c
        eids = np.where(m)[0]
        lo_c, hi_c = _prep_passA(src_all[eids], dest_all[eids],
                                 attention_mask[eids], c)
        coreA.append((eids, lo_c, hi_c))

    def rup(x, m):
        return ((x + m - 1) // m) * m

    ch_lo = rup(max(len(a[1]) for a in coreA), BATCH)
    ch_hi = rup(max(len(a[2]) for a in coreA), BATCH)
    CH_A = ch_lo + ch_hi

    in1 = []
    relayA = []   # per core: global edge id at (passA chunk, partition)
    for c in range(NCORES):
        eids, lo_c, hi_c = coreA[c]
        lo_c = _pad_chunks(list(lo_c), ch_lo, 0)
        hi_c = _pad_chunks(list(hi_c), ch_hi, LO // 128)
        allc = lo_c + hi_c
        qi = np.concatenate([ch[1] for ch in lo_c] +
                            [ch[1] - LO for ch in hi_c])
        ki = np.concatenate([ch[2] for ch in allc])
        srcl = np.stack([ch[1] % 128 for ch in allc], 1).astype(np.float32)
        soff = np.array([[ch[0] * H for ch in allc]], np.int32)
        maskp = np.stack([ch[3] for ch in allc], 1).reshape(128, CH_A * H)
        gids = np.stack([ch[4] for ch in allc], 1)  # [128, CH_A] local eids
        gmap = np.full((128, CH_A), -1, np.int64)
        valid = gids >= 0
        gmap[valid] = eids[gids[valid]]
        relayA.append(gmap)
        base = c * RPC
        nfTk = np.zeros((128, KT * 128), np.float32)
        nfTk[:, :RPC] = nfT[:, base:base + RPC]
        in1.append({
            "nfT": nfT, "nfTk": nfTk, "WqT": WqT, "WkT": WkT,
            "bqk": np.concatenate([bq4, bk])[None, :].astype(np.float32),
            "qidx": _wrap_idx(qi.astype(np.int16)),
            "kidx": _wrap_idx(ki.astype(np.int16)),
            "srcloc": np.ascontiguousarray(srcl),
            "segoff": soff,
            "maskp": np.ascontiguousarray(maskp.astype(np.float32)),
        })

    emulate = bool(os.environ.get("KERNEL_EMULATE"))
    ncores_run = int(os.environ.get("KERNEL_NCORES", NCORES))
    if emulate:
        r1 = _emulate_run1(in1, ch_lo, ch_hi)
    else:
        nc1 = _gen_run1(ch_lo, ch_hi)
        r1 = run_bass_kernel_spmd(nc1, in1[:ncores_run],
                                  core_ids=list(range(ncores_run))).results
        if os.environ.get("KERNEL_RUN1_ONLY"):
            print("RUN1 OK")
            return np.zeros((N, F), np.float32)
        if ncores_run < NCORES:
            r1 = list(r1) + [r1[0]] * (NCORES - ncores_run)
    seg = r1[0]["segout"]  # same on all cores post-allreduce

    # global per-edge ex [E, H] from per-core exout
    ex_edge = np.zeros((E, H), np.float32)
    for c in range(NCORES):
        exo = r1[c]["exout"].reshape(128, CH_A, H)
        gmap = relayA[c]
        v = gmap >= 0
        ex_edge[gmap[v]] = exo[v]

    # ---------- pass-B prep ----------
    coreB = []
    for c in range(NCORES):
        m = (dest_all // RPC) == c
        eids = np.where(m)[0]
        lo_c, hi_c = _prep_passB(src_all[eids], dest_all[eids], c)
        # note: _prep_passB uses global edge ids relative to eids subset
        coreB.append((eids, lo_c, hi_c))
    chB_lo = rup(max(len(b[1]) for b in coreB), BATCH)
    chB_hi = rup(max(len(b[2]) for b in coreB), BATCH)
    CH_B = chB_lo + chB_hi

    in2 = []
    for c in range(NCORES):
        eids, lo_c, hi_c = coreB[c]
        lo_c = _pad_chunksB(list(lo_c), chB_lo, True)
        hi_c = _pad_chunksB(list(hi_c), chB_hi, False)
        allc = lo_c + hi_c
        vi = np.concatenate([ch[1] for ch in lo_c] +
                            [ch[1] - LO for ch in hi_c])
        dloc = np.stack([ch[2] for ch in allc], 1).astype(np.float32)
        ooff = np.array([[ch[0] * 128 for ch in allc]], np.int32)
        exB = np.zeros((128, CH_B, H), np.float32)
        gids = np.stack([ch[3] for ch in allc], 1)
        v = gids >= 0
        exB[v] = ex_edge[eids[gids[v]]]
        in2.append({
            "nfT": nfT, "WvT": WvT, "WoT": WoT,
            "bvo": np.concatenate([bv, bo])[None, :].astype(np.float32),
            "seg": seg,
            "vidx": _wrap_idx(vi.astype(np.int16)),
            "exB": np.ascontiguousarray(exB.reshape(128, CH_B * H)),
            "destloc": np.ascontiguousarray(dloc),
            "outoff": ooff,
        })

    if emulate:
        r2 = _emulate_run2(in2, chB_lo, chB_hi)
    else:
        nc2 = _gen_run2(chB_lo, chB_hi)
        r2 = run_bass_kernel_spmd(nc2, in2, core_ids=list(range(NCORES))).results
    out = np.concatenate([r2[c]["outd"] for c in range(NCORES)], 0)
    return out.astype(np.float32)


# revision 16
# speedup vs baseline: 7833.4995x; 7833.4995x over previous
"""GNN edge-softmax attention kernel for Trainium2 (8 NeuronCores).

Strategy (dest-range sharding, two device passes):
  - Core c owns dest-node range [c*6250, (c+1)*6250); edges assigned by dest.
  - Run 1: project q (full) + k (own range) from host-transposed features;
    pass A over edges sorted by src-tile: gather q[src]/k[dest] rows via
    dma_gather, scores -> ex = exp(q.k/4 + mask - 8)  (constant softmax
    shift; exact softmax invariance), segment-sum ex by src via one-hot
    matmul + dynamic-offset accumulate; AllReduce segsum across cores.
  - Host: permute per-edge ex values from pass-A order to pass-B order
    (pure np.take reordering, no arithmetic).
  - Run 2: vn = (v projection) * (1/segsum) per node; pass B over edges
    sorted by dest-tile (src-lo/hi sections for int16 gather indices):
    gather vn[src], weighted = vn * ex, scatter to dest tiles via one-hot
    matmul + dynamic-offset accumulate; final out = acc @ Wo.T + bo.
"""

import os

import numpy as np

import concourse.bacc as bacc
import concourse.bass as bass
import concourse.mybir as mybir
import concourse.tile as tile
from concourse.bass_utils import run_bass_kernel_spmd
from concourse.masks import make_identity

N = 50000
F = 128
H = 8
D = 16
E = 800000
NCORES = 8
RPC = N // NCORES          # 6250 rows per core
KT = (RPC + 127) // 128    # 49 dest tiles per core (last has 106 rows)
NT = (N + 127) // 128      # 391 src tiles (last has 80 rows)
NPAD = NT * 128            # 50048
LO = 32768                 # int16 index boundary
BATCH = 8                  # chunks per dma_gather call (1024 rows)
MASK_PAD = -60.0
LAST_NC1 = None
LAST_NC2 = None
SHIFT = 8.0

f32 = mybir.dt.float32
i16 = mybir.dt.int16
i32 = mybir.dt.int32
DVE = mybir.EngineType.DVE
ALU = mybir.AluOpType
ACTF = mybir.ActivationFunctionType


def _wrap_idx(flat):
    """int16 row-index array -> dma_gather wrapped layout [128, len/16]."""
    nb = len(flat) // 1024
    out = np.zeros((128, 64 * nb), np.int16)
    for b in range(nb):
        sl = flat[b * 1024:(b + 1) * 1024].reshape(64, 16).T
        out[:, b * 64:(b + 1) * 64] = np.tile(sl, (8, 1))
    return out


def _bcast16(ap):
    """[128, 8] AP -> [128, 8, 16] with stride-0 inner broadcast."""
    return bass.AP(ap.tensor, ap.offset, list(ap.ap) + [[0, D]])


def _gen_run1(ch_lo, ch_hi):
    CH = ch_lo + ch_hi
    nc = bacc.Bacc(None, target_bir_lowering=False, num_swdge_queues=4)
    nfT = nc.dram_tensor("nfT", [128, NPAD], f32, kind="ExternalInput")
    nfTk = nc.dram_tensor("nfTk", [128, KT * 128], f32, kind="ExternalInput")
    WqT = nc.dram_tensor("WqT", [128, 128], f32, kind="ExternalInput")
    WkT = nc.dram_tensor("WkT", [128, 128], f32, kind="ExternalInput")
    bqk = nc.dram_tensor("bqk", [1, 256], f32, kind="ExternalInput")
    qidx = nc.dram_tensor("qidx", [128, CH * 8], i16, kind="ExternalInput")
    kidx = nc.dram_tensor("kidx", [128, CH * 8], i16, kind="ExternalInput")
    srcloc = nc.dram_tensor("srcloc", [128, CH], f32, kind="ExternalInput")
    segoff = nc.dram_tensor("segoff", [1, CH], i32, kind="ExternalInput")
    maskp = nc.dram_tensor("maskp", [128, CH * H], f32, kind="ExternalInput")
    exout = nc.dram_tensor("exout", [128, CH * H], f32, kind="ExternalOutput")
    segout = nc.dram_tensor("segout", [128, NT * H], f32, kind="ExternalOutput")

    qtab = nc.dram_tensor("qtab", [NPAD, F], f32)
    ktab = nc.dram_tensor("ktab", [KT * 128, F], f32)
    seg_in = nc.dram_tensor("seg_in", [128, NT * H], f32)
    seg_cc = nc.dram_tensor("seg_cc", [128, NT * H], f32)

    with tile.TileContext(nc) as tc:
        with (
            tc.tile_pool(name="meta", bufs=1) as mp,
            tc.tile_pool(name="proj", bufs=3) as pp,
            tc.tile_pool(name="gath", bufs=3) as gp,
            tc.tile_pool(name="work", bufs=4) as wp,
            tc.tile_pool(name="ps", bufs=3, space="PSUM") as ps,
            tc.tile_pool(name="pseg", bufs=4, space="PSUM") as pseg,
        ):
            # ---- constants / metadata loads ----
            wq = mp.tile([128, 128], f32)
            wk = mp.tile([128, 128], f32)
            bias = mp.tile([1, 256], f32)
            ones = mp.tile([1, 128], f32)
            iota = mp.tile([128, 128], f32)
            nc.sync.dma_start(out=wq[:], in_=WqT[:])
            nc.sync.dma_start(out=wk[:], in_=WkT[:])
            nc.sync.dma_start(out=bias[:], in_=bqk[:])
            nc.vector.memset(ones[:], 1.0)
            nc.gpsimd.iota(iota[:], pattern=[[1, 128]], base=0,
                           channel_multiplier=0,
                           allow_small_or_imprecise_dtypes=True)

            qidx_sb = mp.tile([128, CH * 8], i16)
            kidx_sb = mp.tile([128, CH * 8], i16)
            srcloc_sb = mp.tile([128, CH], f32)
            segoff_sb = mp.tile([1, CH], i32)
            mask_sb = mp.tile([128, CH * H], f32)
            nc.sync.dma_start(out=qidx_sb[:], in_=qidx[:])
            nc.sync.dma_start(out=kidx_sb[:], in_=kidx[:])
            nc.sync.dma_start(out=srcloc_sb[:], in_=srcloc[:])
            nc.sync.dma_start(out=segoff_sb[:], in_=segoff[:])
            nc.sync.dma_start(out=mask_sb[:], in_=maskp[:])

            # ---- phase 0: q table (full, scaled by 0.25 via host WqT) ----
            for t in range(NT):
                nft = pp.tile([128, 128], f32, tag="nft")
                nc.sync.dma_start(out=nft[:], in_=nfT[:, t * 128:(t + 1) * 128])
                pq = ps.tile([128, 128], f32, tag="pq")
                nc.tensor.matmul(out=pq[:], lhsT=nft[:], rhs=wq[:],
                                 start=True, stop=False)
                nc.tensor.matmul(out=pq[:], lhsT=ones[:], rhs=bias[:, 0:128],
                                 start=False, stop=True)
                qsb = pp.tile([128, 128], f32, tag="qsb")
                nc.scalar.activation(out=qsb[:], in_=pq[:], func=ACTF.Copy)
                nc.sync.dma_start(out=qtab[t * 128:(t + 1) * 128, :], in_=qsb[:])

            # ---- phase 0b: k table (own dest range, via per-core nfTk) ----
            for t in range(KT):
                nft = pp.tile([128, 128], f32, tag="nft")
                nc.sync.dma_start(out=nft[:], in_=nfTk[:, t * 128:(t + 1) * 128])
                pk = ps.tile([128, 128], f32, tag="pq")
                nc.tensor.matmul(out=pk[:], lhsT=nft[:], rhs=wk[:],
                                 start=True, stop=False)
                nc.tensor.matmul(out=pk[:], lhsT=ones[:], rhs=bias[:, 128:256],
                                 start=False, stop=True)
                ksb = pp.tile([128, 128], f32, tag="qsb")
                nc.scalar.activation(out=ksb[:], in_=pk[:], func=ACTF.Copy)
                nc.sync.dma_start(out=ktab[t * 128:(t + 1) * 128, :], in_=ksb[:])

            # ---- pass A ----
            segacc = mp.tile([128, NT * H], f32)
            nc.vector.memset(segacc[:], 0.0)
            ex_sb = mp.tile([128, CH * H], f32)

            nb_lo = ch_lo // BATCH
            nb = CH // BATCH
            for b in range(nb):
                qg = gp.tile([128, BATCH, F], f32, tag="qg")
                kg = gp.tile([128, BATCH, F], f32, tag="kg")
                if b < nb_lo:
                    qsrc = qtab[0:LO, :]
                else:
                    qsrc = qtab[LO:NPAD, :]
                nc.gpsimd.dma_gather(
                    out_ap=qg[:], in_ap=qsrc,
                    idxs_ap=qidx_sb[:, b * 64:(b + 1) * 64],
                    num_idxs=1024, num_idxs_reg=1024, elem_size=F,
                    queue_num=(2 * b) % 4)
                nc.gpsimd.dma_gather(
                    out_ap=kg[:], in_ap=ktab[:],
                    idxs_ap=kidx_sb[:, b * 64:(b + 1) * 64],
                    num_idxs=1024, num_idxs_reg=1024, elem_size=F,
                    queue_num=(2 * b + 1) % 4)
                for j in range(BATCH):
                    c = b * BATCH + j
                    prod = wp.tile([128, F], f32, tag="prod")
                    nc.vector.tensor_tensor(out=prod[:], in0=qg[:, j, :],
                                            in1=kg[:, j, :], op=ALU.mult)
                    nc.vector.tensor_reduce(
                        out=ex_sb[:, c * H:(c + 1) * H],
                        in_=prod[:].rearrange("p (h d) -> p h d", d=D),
                        axis=mybir.AxisListType.X, op=ALU.add)
                # batch-level: add mask, exp
                sl = slice(b * BATCH * H, (b + 1) * BATCH * H)
                nc.vector.tensor_tensor(out=ex_sb[:, sl], in0=ex_sb[:, sl],
                                        in1=mask_sb[:, sl], op=ALU.add)
                nc.scalar.activation(out=ex_sb[:, sl], in_=ex_sb[:, sl],
                                     func=ACTF.Exp)
                for j in range(BATCH):
                    c = b * BATCH + j
                    oh = wp.tile([128, 128], f32, tag="oh")
                    nc.vector.tensor_scalar(
                        out=oh[:], in0=iota[:], scalar1=srcloc_sb[:, c:c + 1],
                        scalar2=None, op0=ALU.is_equal)
                    psg = pseg.tile([128, H], f32, tag="pseg")
                    nc.tensor.matmul(out=psg[:], lhsT=oh[:],
                                     rhs=ex_sb[:, c * H:(c + 1) * H],
                                     start=True, stop=True)
                    regs = nc.alloc_registers(engines=[DVE])
                    nc.vector.reg_load(regs, segoff_sb[0:1, c:c + 1])
                    off = nc.snap(regs, donate=True, min_val=0,
                                  max_val=(NT - 1) * H)
                    nc.vector.tensor_tensor(
                        out=segacc[:, bass.ds(off, H)],
                        in0=segacc[:, bass.ds(off, H)],
                        in1=psg[:], op=ALU.add)

            nc.sync.dma_start(out=exout[:], in_=ex_sb[:])

            # ---- AllReduce segsum ----
            if os.environ.get("KERNEL_NO_CC"):
                nc.sync.dma_start(out=segout[:], in_=segacc[:])
            else:
                nc.sync.dma_start(out=seg_in[:], in_=segacc[:])
                nc.gpsimd.collective_compute(
                    "AllReduce", ALU.add,
                    replica_groups=[list(range(NCORES))],
                    ins=[seg_in[:]], outs=[seg_cc[:]])
                segf = mp.tile([128, NT * H], f32)
                nc.sync.dma_start(out=segf[:], in_=seg_cc[:])
                nc.sync.dma_start(out=segout[:], in_=segf[:])
    nc.compile()
    return nc


def _gen_run2(ch_lo, ch_hi):
    CH = ch_lo + ch_hi
    nc = bacc.Bacc(None, target_bir_lowering=False, num_swdge_queues=4)
    nfT = nc.dram_tensor("nfT", [128, NPAD], f32, kind="ExternalInput")
    WvT = nc.dram_tensor("WvT", [128, 128], f32, kind="ExternalInput")
    WoT = nc.dram_tensor("WoT", [128, 128], f32, kind="ExternalInput")
    bvo = nc.dram_tensor("bvo", [1, 256], f32, kind="ExternalInput")
    seg = nc.dram_tensor("seg", [128, NT * H], f32, kind="ExternalInput")
    vidx = nc.dram_tensor("vidx", [128, CH * 8], i16, kind="ExternalInput")
    exB = nc.dram_tensor("exB", [128, CH * H], f32, kind="ExternalInput")
    destloc = nc.dram_tensor("destloc", [128, CH], f32, kind="ExternalInput")
    outoff = nc.dram_tensor("outoff", [1, CH], i32, kind="ExternalInput")
    outd = nc.dram_tensor("outd", [RPC, F], f32, kind="ExternalOutput")

    vntab = nc.dram_tensor("vntab", [NPAD, F], f32)

    with tile.TileContext(nc) as tc:
        with (
            tc.tile_pool(name="meta", bufs=1) as mp,
            tc.tile_pool(name="proj", bufs=3) as pp,
            tc.tile_pool(name="gath", bufs=3) as gp,
            tc.tile_pool(name="work", bufs=4) as wp,
            tc.tile_pool(name="ps", bufs=2, space="PSUM") as ps,
            tc.tile_pool(name="pmm", bufs=3, space="PSUM") as pmm,
        ):
            wv = mp.tile([128, 128], f32)
            wo = mp.tile([128, 128], f32)
            bias = mp.tile([1, 256], f32)
            ones = mp.tile([1, 128], f32)
            iota = mp.tile([128, 128], f32)
            ident = mp.tile([128, 128], f32)
            nc.sync.dma_start(out=wv[:], in_=WvT[:])
            nc.sync.dma_start(out=wo[:], in_=WoT[:])
            nc.sync.dma_start(out=bias[:], in_=bvo[:])
            nc.vector.memset(ones[:], 1.0)
            nc.gpsimd.iota(iota[:], pattern=[[1, 128]], base=0,
                           channel_multiplier=0,
                           allow_small_or_imprecise_dtypes=True)
            make_identity(nc, ident[:])

            seg_sb = mp.tile([128, NT * H], f32)
            inv_sb = mp.tile([128, NT * H], f32)
            nc.sync.dma_start(out=seg_sb[:], in_=seg[:])
            nc.vector.tensor_scalar(out=seg_sb[:], in0=seg_sb[:],
                                    scalar1=1e-30, scalar2=None, op0=ALU.max)
            nc.vector.reciprocal(out=inv_sb[:], in_=seg_sb[:])

            vidx_sb = mp.tile([128, CH * 8], i16)
            exB_sb = mp.tile([128, CH * H], f32)
            destloc_sb = mp.tile([128, CH], f32)
            outoff_sb = mp.tile([1, CH], i32)
            nc.sync.dma_start(out=vidx_sb[:], in_=vidx[:])
            nc.sync.dma_start(out=exB_sb[:], in_=exB[:])
            nc.sync.dma_start(out=destloc_sb[:], in_=destloc[:])
            nc.sync.dma_start(out=outoff_sb[:], in_=outoff[:])

            # ---- phase V: vn table = (v projection) * inv_seg ----
            for t in range(NT):
                nft = pp.tile([128, 128], f32, tag="nft")
                nc.sync.dma_start(out=nft[:], in_=nfT[:, t * 128:(t + 1) * 128])
                pv = ps.tile([128, 128], f32, tag="pv")
                nc.tensor.matmul(out=pv[:], lhsT=nft[:], rhs=wv[:],
                                 start=True, stop=False)
                nc.tensor.matmul(out=pv[:], lhsT=ones[:], rhs=bias[:, 0:128],
                                 start=False, stop=True)
                vn = pp.tile([128, 128], f32, tag="vn")
                nc.vector.tensor_tensor(
                    out=vn[:].rearrange("p (h d) -> p h d", d=D),
                    in0=pv[:].rearrange("p (h d) -> p h d", d=D),
                    in1=_bcast16(inv_sb[:, t * H:(t + 1) * H]),
                    op=ALU.mult)
                nc.sync.dma_start(out=vntab[t * 128:(t + 1) * 128, :], in_=vn[:])

            # ---- pass B ----
            acc = mp.tile([128, KT * 128], f32)
            nc.vector.memset(acc[:], 0.0)

            nb_lo = ch_lo // BATCH
            nb = CH // BATCH
            for b in range(nb):
                vg = gp.tile([128, BATCH, F], f32, tag="vg")
                vsrc = vntab[0:LO, :] if b < nb_lo else vntab[LO:NPAD, :]
                nc.gpsimd.dma_gather(
                    out_ap=vg[:], in_ap=vsrc,
                    idxs_ap=vidx_sb[:, b * 64:(b + 1) * 64],
                    num_idxs=1024, num_idxs_reg=1024, elem_size=F,
                    queue_num=b % 4)
                for j in range(BATCH):
                    c = b * BATCH + j
                    wt = wp.tile([128, F], f32, tag="wt")
                    nc.vector.tensor_tensor(
                        out=wt[:].rearrange("p (h d) -> p h d", d=D),
                        in0=vg[:, j, :].rearrange("p (h d) -> p h d", d=D),
                        in1=_bcast16(exB_sb[:, c * H:(c + 1) * H]),
                        op=ALU.mult)
                    oh = wp.tile([128, 128], f32, tag="oh")
                    nc.vector.tensor_scalar(
                        out=oh[:], in0=iota[:],
                        scalar1=destloc_sb[:, c:c + 1],
                        scalar2=None, op0=ALU.is_equal)
                    pm = pmm.tile([128, 128], f32, tag="pm")
                    nc.tensor.matmul(out=pm[:], lhsT=oh[:], rhs=wt[:],
                                     start=True, stop=True)
                    regs = nc.alloc_registers(engines=[DVE])
                    nc.vector.reg_load(regs, outoff_sb[0:1, c:c + 1])
                    off = nc.snap(regs, donate=True, min_val=0,
                                  max_val=(KT - 1) * 128)
                    nc.vector.tensor_tensor(
                        out=acc[:, bass.ds(off, 128)],
                        in0=acc[:, bass.ds(off, 128)],
                        in1=pm[:], op=ALU.add)

            # ---- output projection per dest tile ----
            for t in range(KT):
                rows = min(128, RPC - t * 128)
                pt = ps.tile([128, 128], f32, tag="pv")
                nc.tensor.transpose(out=pt[:], in_=acc[:, t * 128:(t + 1) * 128],
                                    identity=ident[:])
                at = pp.tile([128, 128], f32, tag="at")
                nc.scalar.activation(out=at[:], in_=pt[:], func=ACTF.Copy)
                po = ps.tile([128, 128], f32, tag="po")
                nc.tensor.matmul(out=po[:], lhsT=at[:], rhs=wo[:],
                                 start=True, stop=False)
                nc.tensor.matmul(out=po[:], lhsT=ones[:], rhs=bias[:, 128:256],
                                 start=False, stop=True)
                osb = pp.tile([128, 128], f32, tag="osb")
                nc.scalar.activation(out=osb[:], in_=po[:], func=ACTF.Copy)
                nc.sync.dma_start(out=outd[t * 128:t * 128 + rows, :],
                                  in_=osb[:rows, :])
    nc.compile()
    return nc


def _prep_passA(src, dest, mask, core):
    """Returns per-core pass-A metadata dict + bookkeeping for ex relay."""
    base = core * RPC
    dl = dest - base
    st = src // 128
    order = np.argsort(st, kind="stable")
    src_s, dl_s, mask_s, gid_s = src[order], dl[order], mask[order], order
    lo_chunks = []
    hi_chunks = []
    tile_ids = st[order]
    for t in range(NT):
        lo_i = np.searchsorted(tile_ids, t, side="left")
        hi_i = np.searchsorted(tile_ids, t, side="right")
        cnt = hi_i - lo_i
        if cnt == 0:
            continue
        nch = (cnt + 127) // 128
        for ci in range(nch):
            a = lo_i + ci * 128
            bnd = min(a + 128, hi_i)
            n = bnd - a
            cs = np.zeros(128, np.int64)
            cdl = np.zeros(128, np.int64)
            cm = np.full((128, H), MASK_PAD, np.float32)
            cg = np.full(128, -1, np.int64)
            cs[:n] = src_s[a:bnd]
            cs[n:] = t * 128  # pad src within tile (any row)
            cdl[:n] = dl_s[a:bnd]
            cm[:n] = mask_s[a:bnd] - SHIFT
            cg[:n] = gid_s[a:bnd]
            rec = (t, cs, cdl, cm, cg)
            (lo_chunks if t < LO // 128 else hi_chunks).append(rec)
    return lo_chunks, hi_chunks


def _pad_chunks(chunks, target, tpad):
    while len(chunks) % 1 != 0 or len(chunks) < target:
        cs = np.full(128, tpad * 128, np.int64)
        chunks.append((tpad, cs, np.zeros(128, np.int64),
                       np.full((128, H), MASK_PAD, np.float32),
                       np.full(128, -1, np.int64)))
    return chunks


def _prep_passB(src, dest, core):
    base = core * RPC
    dl = dest - base
    t_of = dl // 128
    is_lo = src < LO
    lo_chunks, hi_chunks = [], []
    for sec, m in ((lo_chunks, is_lo), (hi_chunks, ~is_lo)):
        idx = np.where(m)[0]
        order = idx[np.argsort(t_of[idx], kind="stable")]
        tid = t_of[order]
        for t in range(KT):
            a = np.searchsorted(tid, t, side="left")
            bnd = np.searchsorted(tid, t, side="right")
            cnt = bnd - a
            if cnt == 0:
                continue
            for ci in range((cnt + 127) // 128):
                s = a + ci * 128
                e2 = min(s + 128, bnd)
                n = e2 - s
                csrc = np.zeros(128, np.int64)
                cdl = np.zeros(128, np.int64)
                cg = np.full(128, -1, np.int64)
                eids = order[s:e2]
                csrc[:n] = src[eids]
                csrc[n:] = 0 if sec is lo_chunks else LO
                cdl[:n] = dl[eids] % 128
                cg[:n] = eids
                lo_hi = lo_chunks if sec is lo_chunks else hi_chunks
                lo_hi.append((t, csrc, cdl, cg))
    return lo_chunks, hi_chunks


def _pad_chunksB(chunks, target, lo):
    while len(chunks) < target:
        chunks.append((0, np.full(128, 0 if lo else LO, np.int64),
                       np.zeros(128, np.int64), np.full(128, -1, np.int64)))
    return chunks


def _unwrap_idx(arr):
    """Inverse of _wrap_idx: [128, 64*nb] -> flat row indices."""
    nb = arr.shape[1] // 64
    out = np.zeros(nb * 1024, np.int64)
    for b in range(nb):
        sl = arr[:16, b * 64:(b + 1) * 64]          # [16, 64]
        out[b * 1024:(b + 1) * 1024] = sl.T.reshape(-1)
    return out


def _emulate_run1(in1, ch_lo, ch_hi):
    CH = ch_lo + ch_hi
    results = []
    segsum = np.zeros((128, NT * H), np.float32)
    for m in in1:
        qtab = m["nfT"].T @ m["WqT"] + m["bqk"][0, :128]
        ktab = m["nfTk"].T @ m["WkT"] + m["bqk"][0, 128:]
        qi = _unwrap_idx(m["qidx"])
        ki = _unwrap_idx(m["kidx"])
        ex = np.zeros((128, CH, H), np.float32)
        seg = np.zeros((128, NT * H), np.float32)
        for b in range(CH // BATCH):
            qrows = qi[b * 1024:(b + 1) * 1024]
            if b >= ch_lo // BATCH:
                qrows = qrows + LO
            krows = ki[b * 1024:(b + 1) * 1024]
            qg = qtab[qrows].reshape(BATCH, 128, F).transpose(1, 0, 2)
            kg = ktab[krows].reshape(BATCH, 128, F).transpose(1, 0, 2)
            for j in range(BATCH):
                c = b * BATCH + j
                sc = (qg[:, j, :] * kg[:, j, :]).reshape(128, H, D).sum(-1)
                sc = sc + m["maskp"][:, c * H:(c + 1) * H]
                exc = np.exp(sc)
                ex[:, c] = exc
                srcl = m["srcloc"][:, c].astype(np.int64)
                off = m["segoff"][0, c]
                np.add.at(seg[:, off:off + H].T, (slice(None), srcl), exc.T)
        segsum = segsum + seg
        results.append({"exout": ex.reshape(128, CH * H)})
    for r in results:
        r["segout"] = segsum
    return results


def _emulate_run2(in2, ch_lo, ch_hi):
    CH = ch_lo + ch_hi
    results = []
    for m in in2:
        vtab = m["nfT"].T @ m["WvT"] + m["bvo"][0, :128]
        seg = np.maximum(m["seg"], 1e-30)
        inv = 1.0 / seg
        # vn[node] = v[node] * inv[node%128, (node//128)*H + h]
        invn = inv.reshape(128, NT, H).transpose(1, 0, 2).reshape(NPAD, H)
        vn = vtab.reshape(NPAD, H, D) * invn[:, :, None]
        vn = vn.reshape(NPAD, F)
        vi = _unwrap_idx(m["vidx"])
        exB = m["exB"].reshape(128, CH, H)
        acc = np.zeros((128, KT * 128), np.float32)
        for b in range(CH // BATCH):
            vrows = vi[b * 1024:(b + 1) * 1024]
            if b >= ch_lo // BATCH:
                vrows = vrows + LO
            vg = vn[vrows].reshape(BATCH, 128, F).transpose(1, 0, 2)
            for j in range(BATCH):
                c = b * BATCH + j
                wt = (vg[:, j, :].reshape(128, H, D)
                      * exB[:, c][:, :, None]).reshape(128, F)
                dl = m["destloc"][:, c].astype(np.int64)
                off = m["outoff"][0, c]
                np.add.at(acc[:, off:off + 128].T, (slice(None), dl), wt.T)
        # out projection
        accn = acc.reshape(128, KT, 128).transpose(1, 0, 2).reshape(KT * 128, F)
        out = accn @ m["WoT"] + m["bvo"][0, 128:]
        results.append({"outd": out[:RPC].astype(np.float32)})
    return results


def kernel(node_features, edge_index, attention_mask, Wq, bq, Wk, bk,
           Wv, bv, Wo, bo):
    node_features = np.asarray(node_features, np.float32)
    edge_index = np.asarray(edge_index)
    attention_mask = np.asarray(attention_mask, np.float32)
    Wq, bq = np.asarray(Wq, np.float32), np.asarray(bq, np.float32)
    Wk, bk = np.asarray(Wk, np.float32), np.asarray(bk, np.float32)
    Wv, bv = np.asarray(Wv, np.float32), np.asarray(bv, np.float32)
    Wo, bo = np.asarray(Wo, np.float32), np.asarray(bo, np.float32)

    src_all = edge_index[0].astype(np.int64)
    dest_all = edge_index[1].astype(np.int64)

    nfT = np.zeros((128, NPAD), np.float32)
    nfT[:, :N] = node_features.T
    WqT = np.ascontiguousarray(Wq.T) * 0.25  # fold 1/sqrt(D)
    bq4 = bq * 0.25
    WkT = np.ascontiguousarray(Wk.T)
    WvT = np.ascontiguousarray(Wv.T)
    WoT = np.ascontiguousarray(Wo.T)

    # ---------- per-core edge partition & pass-A prep ----------
    coreA = []
    for c in range(NCORES):
        m = (dest_all // RPC) == c
        eids = np.where(m)[0]
        lo_c, hi_c = _prep_passA(src_all[eids], dest_all[eids],
                                 attention_mask[eids], c)
        coreA.append((eids, lo_c, hi_c))

    def rup(x, m):
        return ((x + m - 1) // m) * m

    ch_lo = rup(max(len(a[1]) for a in coreA), BATCH)
    ch_hi = rup(max(len(a[2]) for a in coreA), BATCH)
    CH_A = ch_lo + ch_hi

    in1 = []
    relayA = []   # per core: global edge id at (passA chunk, partition)
    for c in range(NCORES):
        eids, lo_c, hi_c = coreA[c]
        lo_c = _pad_chunks(list(lo_c), ch_lo, 0)
        hi_c = _pad_chunks(list(hi_c), ch_hi, LO // 128)
        allc = lo_c + hi_c
        qi = np.concatenate([ch[1] for ch in lo_c] +
                            [ch[1] - LO for ch in hi_c])
        ki = np.concatenate([ch[2] for ch in allc])
        srcl = np.stack([ch[1] % 128 for ch in allc], 1).astype(np.float32)
        soff = np.array([[ch[0] * H for ch in allc]], np.int32)
        maskp = np.stack([ch[3] for ch in allc], 1).reshape(128, CH_A * H)
        gids = np.stack([ch[4] for ch in allc], 1)  # [128, CH_A] local eids
        gmap = np.full((128, CH_A), -1, np.int64)
        valid = gids >= 0
        gmap[valid] = eids[gids[valid]]
        relayA.append(gmap)
        base = c * RPC
        nfTk = np.zeros((128, KT * 128), np.float32)
        nfTk[:, :RPC] = nfT[:, base:base + RPC]
        in1.append({
            "nfT": nfT, "nfTk": nfTk, "WqT": WqT, "WkT": WkT,
            "bqk": np.concatenate([bq4, bk])[None, :].astype(np.float32),
            "qidx": _wrap_idx(qi.astype(np.int16)),
            "kidx": _wrap_idx(ki.astype(np.int16)),
            "srcloc": np.ascontiguousarray(srcl),
            "segoff": soff,
            "maskp": np.ascontiguousarray(maskp.astype(np.float32)),
        })

    emulate = bool(os.environ.get("KERNEL_EMULATE"))
    ncores_run = int(os.environ.get("KERNEL_NCORES", NCORES))
    global LAST_NC1, LAST_NC2

    if emulate:
        r1 = _emulate_run1(in1, ch_lo, ch_hi)
    else:
        nc1 = _gen_run1(ch_lo, ch_hi)
        LAST_NC1 = nc1
        r1 = run_bass_kernel_spmd(nc1, in1[:ncores_run],
                                  core_ids=list(range(ncores_run))).results
        if os.environ.get("KERNEL_RUN1_ONLY"):
            print("RUN1 OK")
            return np.zeros((N, F), np.float32)
        if ncores_run < NCORES:
            r1 = list(r1) + [r1[0]] * (NCORES - ncores_run)
    seg = r1[0]["segout"]  # same on all cores post-allreduce

    # global per-edge ex [E, H] from per-core exout
    ex_edge = np.zeros((E, H), np.float32)
    for c in range(NCORES):
        exo = r1[c]["exout"].reshape(128, CH_A, H)
        gmap = relayA[c]
        v = gmap >= 0
        ex_edge[gmap[v]] = exo[v]

    # ---------- pass-B prep ----------
    coreB = []
    for c in range(NCORES):
        m = (dest_all // RPC) == c
        eids = np.where(m)[0]
        lo_c, hi_c = _prep_passB(src_all[eids], dest_all[eids], c)
        # note: _prep_passB uses global edge ids relative to eids subset
        coreB.append((eids, lo_c, hi_c))
    chB_lo = rup(max(len(b[1]) for b in coreB), BATCH)
    chB_hi = rup(max(len(b[2]) for b in coreB), BATCH)
    CH_B = chB_lo + chB_hi

    in2 = []
    for c in range(NCORES):
        eids, lo_c, hi_c = coreB[c]
        lo_c = _pad_chunksB(list(lo_c), chB_lo, True)
        hi_c = _pad_chunksB(list(hi_c), chB_hi, False)
        allc = lo_c + hi_c
        vi = np.concatenate([ch[1] for ch in lo_c] +
                            [ch[1] - LO for ch in hi_c])
        dloc = np.stack([ch[2] for ch in allc], 1).astype(np.float32)
        ooff = np.array([[ch[0] * 128 for ch in allc]], np.int32)
        exB = np.zeros((128, CH_B, H), np.float32)
        gids = np.stack([ch[3] for ch in allc], 1)
        v = gids >= 0
        exB[v] = ex_edge[eids[gids[v]]]
        in2.append({
            "nfT": nfT, "WvT": WvT, "WoT": WoT,
            "bvo": np.concatenate([bv, bo])[None, :].astype(np.float32),
            "seg": seg,
            "vidx": _wrap_idx(vi.astype(np.int16)),
            "exB": np.ascontiguousarray(exB.reshape(128, CH_B * H)),
            "destloc": np.ascontiguousarray(dloc),
            "outoff": ooff,
        })

    if emulate:
        r2 = _emulate_run2(in2, chB_lo, chB_hi)
    else:
        nc2 = _gen_run2(chB_lo, chB_hi)
        LAST_NC2 = nc2
        r2 = run_bass_kernel_spmd(nc2, in2, core_ids=list(range(NCORES))).results
    out = np.concatenate([r2[c]["outd"] for c in range(NCORES)], 0)
    return out.astype(np.float32)
